# revision 13
# baseline (speedup 1.0000x reference)
"""DisattentionFormerV2 Trainium2 kernel (8 NeuronCores, SPMD).

Strategy:
  - Projections: token-sharded (core c -> batch c//2, seq-half c%2, 1024 tokens).
    All projection matmuls in fp16, rmsnorm/l2norm in fp32 where it matters.
  - One AllGather distributes k^T / q^T / (v/8)^T (fp16) to every core.
  - Scan (32 chunks): Megatron-TP over the memory-MLP feature dim.
    Each core owns a 128-row shard of W0 and a 128-col shard of W1.
    One fp16 AllReduce of (pred - v) per step is the only per-step collective.
    Weight recurrence uses the S-free two-term form:
        W_{t+1} = c1_t W_t + c2_t W_{t-1} - theta_t g_t
    with the gradient matmuls accumulating -theta*(2/numel)*g directly in PSUM
    (scalar folded into the W1N / a_tok fp16 casts).
  - Retrieval partials accumulate per-core; one ReduceScatter at the end;
    output projection Wo token-sharded.
"""
import os
import sys

for _p in ("/opt/trn_rl_repo", "/root/.axon_site/_ro/trn_rl_repo"):
    if os.path.isdir(_p) and _p not in sys.path:
        sys.path.insert(0, _p)

import numpy as np
import ml_dtypes

import concourse.bass as bass
import concourse.mybir as mybir
import concourse.tile as tile
from concourse import bacc
from concourse.bass_utils import run_bass_kernel_spmd

F32 = mybir.dt.float32
F16 = mybir.dt.float16
AF = mybir.ActivationFunctionType
OP = mybir.AluOpType

B, S, D, CH = 4, 2048, 1024, 64
N = S // CH  # 32 chunks
TOK = B * CH  # 256 tokens per chunk
MEM_LR, MEM_MOM, MEM_DECAY = 0.1, 0.9, 0.01
EPS = 1.1920929e-07
NUMEL = float(B * CH * D)
NC = 8
RG = [list(range(NC))]
NSTEPS = int(os.environ.get("DISA_NSTEPS", N))  # debug knob (32 = full)


def build(nc: bass.Bass):
    # ---------------- parameters ----------------
    xs_p = nc.declare_dram_parameter("xs", [1024, 1024], F32, isOutput=False)
    mb_p = nc.declare_dram_parameter("mb", [1024, 1024], F16, isOutput=False)
    wkT_p = nc.declare_dram_parameter("wkT", [1024, 1024], F16, isOutput=False)
    wqT_p = nc.declare_dram_parameter("wqT", [1024, 1024], F16, isOutput=False)
    wvT_p = nc.declare_dram_parameter("wvT", [1024, 1024], F16, isOutput=False)
    woT_p = nc.declare_dram_parameter("woT", [1024, 1024], F16, isOutput=False)
    wgT_p = nc.declare_dram_parameter("wgT", [3 * 1024, 1024], F16, isOutput=False)
    gb_p = nc.declare_dram_parameter("gb", [1, 4096], F32, isOutput=False)
    srow_p = nc.declare_dram_parameter("srow", [4, 1], F32, isOutput=False)
    ones1_p = nc.declare_dram_parameter("ones1", [1, 128], F32, isOutput=False)
    cmask_p = nc.declare_dram_parameter("cmask", [128, 2], F16, isOutput=False)
    eye_p = nc.declare_dram_parameter("eye", [128, 128], F16, isOutput=False)
    eyef_p = nc.declare_dram_parameter("eyef", [128, 128], F32, isOutput=False)
    gcol_p = nc.declare_dram_parameter("gcol", [128, 16], F32, isOutput=False)
    w0t0_p = nc.declare_dram_parameter("w0t0", [128, 1024], F32, isOutput=False)
    v0_p = nc.declare_dram_parameter("v0", [128, 1024], F32, isOutput=False)
    out_p = nc.declare_dram_parameter("out", [1024, 1024], F32, isOutput=True)

    with tile.TileContext(nc) as tc:
        with (
            tc.tile_pool(name="dram", bufs=1, space="DRAM") as dpool,
            tc.tile_pool(name="drcc", bufs=2, space="DRAM") as dcc,
            tc.tile_pool(name="const", bufs=1) as cpool,
            tc.tile_pool(name="state", bufs=1) as spool,
        ):
            # DRAM internals
            pk = dpool.tile([3 * 1024, 1024], F16, tag="pk")
            agb = dpool.tile([NC * 3 * 1024, 1024], F16, tag="agb")
            cmb = dpool.tile([16, 1024], F16, tag="cmb")
            cmg = dpool.tile([NC * 16, 1024], F16, tag="cmg")
            rbuf = dpool.tile([NC * 1024, 1024], F16, tag="rbuf")
            rsh = dpool.tile([1024, 1024], F16, tag="rsh")

            # constants in SBUF
            eye = cpool.tile([128, 128], F16, tag="eye")
            eyef = cpool.tile([128, 128], F32, tag="eyef")
            ones1 = cpool.tile([1, 128], F32, tag="ones1")
            cmask = cpool.tile([128, 2], F16, tag="cmask")
            gcol = cpool.tile([128, 16], F32, tag="gcol")
            srow = cpool.tile([4, 1], F32, tag="srow")
            gbsb = cpool.tile([1, 4096], F32, tag="gbsb")
            nc.sync.dma_start(out=eye[:], in_=eye_p[:])
            nc.sync.dma_start(out=eyef[:], in_=eyef_p[:])
            nc.sync.dma_start(out=ones1[:], in_=ones1_p[:])
            nc.sync.dma_start(out=cmask[:], in_=cmask_p[:])
            nc.sync.dma_start(out=gcol[:], in_=gcol_p[:])
            nc.sync.dma_start(out=srow[:], in_=srow_p[:])
            nc.sync.dma_start(out=gbsb[:], in_=gb_p[:])

            # persistent state
            w0A = spool.tile([128, 1024], F32, tag="w0A")
            w0B = spool.tile([128, 1024], F32, tag="w0B")
            vA = spool.tile([128, 1024], F32, tag="vA")
            vB = spool.tile([128, 1024], F32, tag="vB")
            w0h = spool.tile([128, 1024], F16, tag="w0h")
            vh = spool.tile([128, 1024], F16, tag="vh")
            w1n = spool.tile([128, 1024], F16, tag="w1n")
            bc = spool.tile([128, 96], F32, tag="bc")

            epsb = cpool.tile([128, 1], F32, tag="epsb")
            nc.vector.memset(epsb[:], EPS)
            nc.sync.dma_start(out=w0A[:], in_=w0t0_p[:])
            nc.sync.dma_start(out=vA[:], in_=v0_p[:])
            nc.vector.tensor_copy(w0B[:], w0A[:])
            nc.vector.tensor_copy(vB[:], vA[:])
            nc.scalar.copy(w0h[:], w0A[:])
            nc.scalar.copy(vh[:], vA[:])

            rearr = lambda p: p[:].rearrange("(f dl) c -> dl f c", dl=128)

            # ====== PHASES 1+2 share a scoped pool (freed before the scan) ======
            with (
                tc.tile_pool(name="proj", bufs=1) as pj,
                tc.tile_pool(name="p1", bufs=2) as p1,
                tc.tile_pool(name="ps1", bufs=2, space="PSUM") as ps1,
                tc.tile_pool(name="pst", bufs=2, space="PSUM") as pst,
            ):
                # big weight tiles (SBUF, packed [d_lo, f*1024 + col])
                mbsb = pj.tile([128, 8192], F16, tag="mbsb")
                wksb = pj.tile([128, 8192], F16, tag="wkq")
                xsT = pj.tile([128, 8192], F16, tag="xsT")
                xrT = pj.tile([128, 8192], F16, tag="xrT")
                yT = pj.tile([128, 8192], F16, tag="yT")
                cm16 = pj.tile([2, 8192], F16, tag="cm16")
                nc.sync.dma_start(out=mbsb[:].rearrange("p (f c) -> p f c", f=8), in_=rearr(mb_p))
                nc.sync.dma_start(out=wksb[:].rearrange("p (f c) -> p f c", f=8), in_=rearr(wkT_p))

                # -------- phase 1: norms, transposes, cm --------
                for i in range(8):
                    xt = p1.tile([128, 1024], F32, tag="xt")
                    xth = p1.tile([128, 1024], F16, tag="xth")
                    scr = p1.tile([128, 1024], F16, tag="scr")
                    ssq = p1.tile([128, 1], F32, tag="ssq")
                    rms = p1.tile([128, 1], F32, tag="rms")
                    rin = p1.tile([128, 1], F32, tag="rin")
                    ntile = p1.tile([128, 1024], F32, tag="ntile")
                    nc.sync.dma_start(out=xt[:], in_=xs_p[i * 128:(i + 1) * 128, :])
                    nc.scalar.copy(xth[:], xt[:])
                    cmp_ = ps1.tile([2, 1024], F32, tag="yp")
                    for half in range(2):
                        nc.tensor.matmul(
                            cmp_[:, half * 512:(half + 1) * 512],
                            cmask[:], xth[:, half * 512:(half + 1) * 512],
                            start=True, stop=True)
                    nc.scalar.activation(cm16[:, i * 1024:(i + 1) * 1024], cmp_[:],
                                         AF.Copy, scale=1.0 / 64.0)
                    nc.scalar.activation(scr[:], xt[:], AF.Square, accum_out=ssq[:])
                    nc.scalar.activation(rms[:], ssq[:], AF.Sqrt, scale=1.0 / 1024.0,
                                         bias=epsb[:])
                    nc.vector.reciprocal(rin[:], rms[:])
                    nc.scalar.activation(ntile[:], xt[:], AF.Copy, scale=rin[:])
                    for f in range(8):
                        tp = pst.tile([128, 128], F32, tag="tp1")
                        nc.tensor.transpose(tp[:], ntile[:, f * 128:(f + 1) * 128], eyef[:])
                        nc.scalar.activation(
                            xsT[:, f * 1024 + i * 128: f * 1024 + (i + 1) * 128],
                            tp[:], AF.Copy, scale=gcol[:, f:f + 1])
                        nc.scalar.activation(
                            xrT[:, f * 1024 + i * 128: f * 1024 + (i + 1) * 128],
                            tp[:], AF.Copy, scale=gcol[:, 8 + f:8 + f + 1])
                nc.sync.dma_start(out=cmb[:].rearrange("(i j) d -> j i d", j=2),
                                  in_=cm16[:].rearrange("p (i d) -> p i d", i=8))
                nc.gpsimd.collective_compute(
                    "AllGather", OP.bypass, replica_groups=RG,
                    ins=[cmb.opt()], outs=[cmg.opt()])

                # -------- phase 2: projections --------
                p2 = p1
                ps2 = ps1
                # y^T = Mmet^T-free bmm: yT[f] accumulates over d-chunks
                for f in range(8):
                    yp = ps2.tile([128, 1024], F32, tag="yp")
                    for dt in range(8):
                        for nh in range(2):
                            nc.tensor.matmul(
                                yp[:, nh * 512:(nh + 1) * 512],
                                mbsb[:, dt * 1024 + f * 128: dt * 1024 + (f + 1) * 128],
                                xsT[:, dt * 1024 + nh * 512: dt * 1024 + (nh + 1) * 512],
                                start=(dt == 0), stop=(dt == 7))
                    nc.scalar.copy(yT[:, f * 1024:(f + 1) * 1024], yp[:])
                # v8T = silu(Wv @ xsT) / 8  -> straight to pk
                wvsb = pj.tile([128, 8192], F16, tag="mbsb")
                nc.sync.dma_start(out=wvsb[:].rearrange("p (f c) -> p f c", f=8), in_=rearr(wvT_p))
                for f in range(8):
                    vp = ps2.tile([128, 1024], F32, tag="yp")
                    for dt in range(8):
                        for nh in range(2):
                            nc.tensor.matmul(
                                vp[:, nh * 512:(nh + 1) * 512],
                                wvsb[:, dt * 1024 + f * 128: dt * 1024 + (f + 1) * 128],
                                xsT[:, dt * 1024 + nh * 512: dt * 1024 + (nh + 1) * 512],
                                start=(dt == 0), stop=(dt == 7))
                    vsl = p2.tile([128, 1024], F16, tag="vsl")
                    v8 = p2.tile([128, 1024], F16, tag="v8")
                    nc.scalar.activation(vsl[:], vp[:], AF.Silu)
                    nc.vector.tensor_scalar_mul(v8[:], vsl[:], 0.125)
                    nc.sync.dma_start(
                        out=pk[2048 + f * 128: 2048 + (f + 1) * 128, :]
                        .rearrange("dl (m c) -> dl m c", m=8),
                        in_=v8[:].rearrange("p (m c) -> p m c", m=8))
                # k and q paths: token-major raw, l2norm, transpose to [d, t]
                for (src, row0) in ((yT, 0), (xrT, 1024)):
                    if row0 == 0:
                        wsb = wksb
                    else:
                        wsb = pj.tile([128, 8192], F16, tag="wkq")
                        nc.sync.dma_start(
                            out=wsb[:].rearrange("p (f c) -> p f c", f=8),
                            in_=rearr(wqT_p))
                    for m in range(8):
                        kp = ps2.tile([128, 1024], F32, tag="yp")
                        for dt in range(8):
                            for nh in range(2):
                                nc.tensor.matmul(
                                    kp[:, nh * 512:(nh + 1) * 512],
                                    src[:, dt * 1024 + m * 128: dt * 1024 + (m + 1) * 128],
                                    wsb[:, dt * 1024 + nh * 512: dt * 1024 + (nh + 1) * 512],
                                    start=(dt == 0), stop=(dt == 7))
                        kraw = p2.tile([128, 1024], F16, tag="kraw")
                        sqs = p2.tile([128, 1024], F16, tag="sqs")
                        nr2 = p2.tile([128, 1], F32, tag="nr2")
                        nr = p2.tile([128, 1], F32, tag="nr")
                        nrm = p2.tile([128, 1], F32, tag="nrm")
                        nri = p2.tile([128, 1], F32, tag="nri")
                        knrm = p2.tile([128, 1024], F16, tag="knrm")
                        nc.scalar.activation(kraw[:], kp[:], AF.Silu)
                        nc.scalar.activation(sqs[:], kraw[:], AF.Square, accum_out=nr2[:])
                        nc.scalar.activation(nr[:], nr2[:], AF.Sqrt)
                        nc.vector.tensor_scalar_max(nrm[:], nr[:], 1e-12)
                        nc.vector.reciprocal(nri[:], nrm[:])
                        nc.scalar.activation(knrm[:], kraw[:], AF.Copy, scale=nri[:])
                        for f in range(8):
                            tp = ps2.tile([128, 128], F16, tag="tp2")
                            nc.tensor.transpose(tp[:], knrm[:, f * 128:(f + 1) * 128], eye[:])
                            ksl = p2.tile([128, 128], F16, tag="ksl")
                            nc.scalar.copy(ksl[:], tp[:])
                            nc.sync.dma_start(
                                out=pk[row0 + f * 128: row0 + (f + 1) * 128,
                                       m * 128:(m + 1) * 128],
                                in_=ksl[:])

            nc.gpsimd.collective_compute(
                "AllGather", OP.bypass, replica_groups=RG,
                ins=[pk.opt()], outs=[agb.opt()])

            # ============ PHASE 3a: gates ============
            with (
                tc.tile_pool(name="pg", bufs=1) as pg,
                tc.tile_pool(name="psg", bufs=1, space="PSUM") as psg,
                tc.tile_pool(name="psgt", bufs=2, space="PSUM") as psgt,
            ):
                cmsb2 = pg.tile([128, 1024], F16, tag="cmsb2")
                nc.sync.dma_start(out=cmsb2[:], in_=cmg[:])
                cmT = pg.tile([128, 1024], F16, tag="cmT")
                for f in range(8):
                    tp = psgt.tile([128, 128], F16, tag="tpg")
                    nc.tensor.transpose(tp[:], cmsb2[:, f * 128:(f + 1) * 128], eye[:])
                    nc.scalar.copy(cmT[:, f * 128:(f + 1) * 128], tp[:])
                wgsb = pg.tile([128, 8192], F16, tag="wgsb")
                sigs = pg.tile([128, 4], F32, tag="sigs")
                nc.vector.memset(sigs[:], 0.0)
                for g in range(3):
                    nc.sync.dma_start(
                        out=wgsb[:].rearrange("p (f c) -> p f c", f=8),
                        in_=wgT_p[g * 1024:(g + 1) * 1024, :]
                        .rearrange("(f dl) c -> dl f c", dl=128))
                    zg = psg.tile([128, 1024], F32, tag="zg")
                    for dt in range(8):
                        for nh in range(2):
                            nc.tensor.matmul(
                                zg[:, nh * 512:(nh + 1) * 512],
                                cmT[:, dt * 128:(dt + 1) * 128],
                                wgsb[:, dt * 1024 + nh * 512: dt * 1024 + (nh + 1) * 512],
                                start=(dt == 0), stop=False)
                    for nh in range(2):
                        nc.tensor.matmul(
                            zg[:, nh * 512:(nh + 1) * 512],
                            ones1[:], gbsb[0:1, g * 1024 + nh * 512: g * 1024 + (nh + 1) * 512],
                            start=False, stop=True)
                    sig = pg.tile([128, 1024], F32, tag="sig")
                    nc.scalar.activation(sig[:], zg[:], AF.Sigmoid,
                                         accum_out=sigs[:, g:g + 1])
                stp = psg.tile([4, 128], F32, tag="stp")
                nc.tensor.transpose(stp[:], sigs[:], eyef[:])
                sigT = pg.tile([4, 128], F32, tag="sigT")
                nc.vector.tensor_copy(sigT[:], stp[:])
                # flatten gate rows onto partition 0 (engines can't address base=1,2)
                grow = pg.tile([1, 512], F32, tag="grow")
                nc.sync.dma_start(out=grow[:].rearrange("p (g r) -> p g r", g=4),
                                  in_=sigT[:])
                G = pg.tile([1, 96], F32, tag="G")  # [alpha|theta|eta] on free dim
                t1 = pg.tile([1, 32], F32, tag="t1")
                t2 = pg.tile([1, 32], F32, tag="t2")
                gscale = (MEM_DECAY / 4096.0, MEM_LR / 4096.0, MEM_MOM / 4096.0)
                for g in range(3):
                    nc.vector.tensor_add(t1[:], grow[:, g * 128:g * 128 + 32],
                                         grow[:, g * 128 + 32:g * 128 + 64])
                    nc.vector.tensor_add(t2[:], grow[:, g * 128 + 64:g * 128 + 96],
                                         grow[:, g * 128 + 96:g * 128 + 128])
                    nc.vector.tensor_add(G[:, g * 32:(g + 1) * 32], t1[:], t2[:])
                    nc.vector.tensor_scalar_mul(G[:, g * 32:(g + 1) * 32],
                                                G[:, g * 32:(g + 1) * 32], gscale[g])
                al, th, et = (G[:, 0:32], G[:, 32:64], G[:, 64:96])
                row96 = pg.tile([1, 96], F32, tag="row96")
                apv = pg.tile([1, 32], F32, tag="apv")
                tmp = pg.tile([1, 32], F32, tag="tmpg")
                # c1 = 1 - alpha + eta ; c1[0] = 1 - alpha[0]
                nc.vector.tensor_sub(row96[:, 0:32], et, al)
                nc.vector.tensor_scalar_add(row96[:, 0:32], row96[:, 0:32], 1.0)
                nc.vector.tensor_scalar(row96[:, 0:1], G[:, 0:1], -1.0, 1.0,
                                        OP.mult, OP.add)
                # c2 = -eta*(1-alpha_prev); c2[0] = 0
                nc.vector.memset(apv[:], 0.0)
                nc.vector.tensor_copy(apv[:, 1:32], G[:, 0:31])
                nc.vector.tensor_scalar(tmp[:], apv[:], -1.0, 1.0, OP.mult, OP.add)
                nc.vector.tensor_mul(row96[:, 32:64], et, tmp[:])
                nc.vector.tensor_scalar_mul(row96[:, 32:64], row96[:, 32:64], -1.0)
                nc.vector.memset(row96[:, 32:33], 0.0)
                # s = -theta * 2/numel
                nc.vector.tensor_scalar_mul(row96[:, 64:96], th, -2.0 / NUMEL)
                bcp = psg.tile([128, 96], F32, tag="bcp")
                nc.tensor.matmul(bcp[:], ones1[:], row96[:], start=True, stop=True)
                nc.vector.tensor_copy(bc[:], bcp[:])
                # W1N for step 0
                for f2 in range(8):
                    tp = psgt.tile([128, 128], F16, tag="tpg")
                    nc.tensor.transpose(tp[:], vh[:, f2 * 128:(f2 + 1) * 128], eye[:])
                    nc.scalar.activation(w1n[:, f2 * 128:(f2 + 1) * 128], tp[:],
                                         AF.Copy, scale=bc[:, 64:65])

            # ============ PHASE 3b: the scan ============
            agb3 = agb[:].rearrange("(s r) c -> s r c", r=3 * 1024)
            rbuf3 = rbuf[:].rearrange("(s r) c -> s r c", r=1024)

            def fetch_chunk(pool, t):
                ktc = pool.tile([128, 2048], F16, tag="ktc")
                qtc = pool.tile([128, 2048], F16, tag="qtc")
                v8c = pool.tile([128, 2048], F16, tag="v8c")
                ktok = pool.tile([128, 2048], F16, tag="ktok")
                h, tl = t // 16, t % 16
                for b in range(4):
                    s_b = 2 * b + h
                    for (buf, dst) in ((0, ktc), (1, qtc), (2, v8c)):
                        src = (agb3[s_b, buf * 1024:(buf + 1) * 1024,
                                    64 * tl:64 * (tl + 1)]
                               .rearrange("(f dl) c -> dl f c", dl=128))
                        nc.sync.dma_start(
                            out=dst[:, b * 512:(b + 1) * 512]
                            .rearrange("p (f c) -> p f c", f=8),
                            in_=src)
                    nc.sync.dma_start_transpose(
                        out=ktok[64 * (b % 2):64 * (b % 2) + 64,
                                 (b // 2) * 1024:(b // 2 + 1) * 1024],
                        in_=agb3[s_b, 0:1024, 64 * tl:64 * (tl + 1)])
                return ktc, qtc, v8c, ktok

            # chunk slice helper: [128, (b,f,c)] -> [128, 4, 64] for chunk-major f
            def csl(tile_, f):
                return tile_[:].rearrange("p (b f c) -> p b f c", b=4, f=8)[:, :, f, :]

            w_cur, w_prev = w0A, w0B
            v_cur, v_prev = vA, vB
            with (
                tc.tile_pool(name="ck", bufs=2) as ckp,
                tc.tile_pool(name="act", bufs=2) as ap,
                tc.tile_pool(name="psz", bufs=1, space="PSUM") as psz,
                tc.tile_pool(name="psp", bufs=1, space="PSUM") as psp,
                tc.tile_pool(name="psw", bufs=1, space="PSUM") as psw,
                tc.tile_pool(name="psv", bufs=1, space="PSUM") as psv,
                tc.tile_pool(name="psd", bufs=1, space="PSUM") as psd,
            ):
                for t in range(NSTEPS):
                    ktc, qtc, v8c, ktok = fetch_chunk(ckp, t)
                    last = (t == NSTEPS - 1)
                    if not last:
                        # ---- forward on k: launch the AllReduce ASAP
                        zk = psz.tile([128, 256], F32, tag="zx")
                        for f in range(8):
                            nc.tensor.matmul(zk[:], w0h[:, f * 128:(f + 1) * 128],
                                             csl(ktc, f), start=(f == 0), stop=(f == 7))
                        akT = ap.tile([128, 256], F16, tag="akT")
                        sd = ap.tile([128, 256], F16, tag="sd")
                        nc.scalar.activation(akT[:], zk[:], AF.Silu)
                        nc.scalar.activation(sd[:], zk[:], AF.Derivative_silu)
                        arin = ap.tile([128, 2048], F16, tag="arin")
                        for half in range(2):
                            pp = psp.tile([128, 1024], F32, tag="pp")
                            for j in range(4):
                                f2 = half * 4 + j
                                nc.tensor.matmul(pp[:, j * 256:(j + 1) * 256],
                                                 vh[:, f2 * 128:(f2 + 1) * 128],
                                                 akT[:], start=True, stop=True)
                                nc.vector.tensor_sub(
                                    arin[:, f2 * 256:(f2 + 1) * 256],
                                    pp[:, j * 256:(j + 1) * 256], csl(v8c, f2))
                        cci = dcc.tile([128, 2048], F16, tag="cci")
                        cco = dcc.tile([128, 2048], F16, tag="cco")
                        nc.sync.dma_start(out=cci[:], in_=arin[:])
                        nc.gpsimd.collective_compute(
                            "AllReduce", OP.add, replica_groups=RG,
                            ins=[cci.opt()], outs=[cco.opt()])
                    # ---- retrieve (fills the AllReduce latency window)
                    zq = psz.tile([128, 256], F32, tag="zx")
                    for f in range(8):
                        nc.tensor.matmul(zq[:], w0h[:, f * 128:(f + 1) * 128],
                                         csl(qtc, f), start=(f == 0), stop=(f == 7))
                    aq = ap.tile([128, 256], F16, tag="aq")
                    nc.scalar.activation(aq[:], zq[:], AF.Silu)
                    rT = ap.tile([128, 2048], F16, tag="rT")
                    for half in range(2):
                        rr = psp.tile([128, 1024], F32, tag="pp")
                        for j in range(4):
                            f2 = half * 4 + j
                            nc.tensor.matmul(rr[:, j * 256:(j + 1) * 256],
                                             vh[:, f2 * 128:(f2 + 1) * 128],
                                             aq[:], start=True, stop=True)
                            nc.scalar.copy(
                                rT[:, f2 * 256:(f2 + 1) * 256],
                                rr[:, j * 256:(j + 1) * 256])
                    h, tl = t // 16, t % 16
                    for b in range(4):
                        nc.sync.dma_start(
                            out=rbuf3[2 * b + h].rearrange("(f dl) c -> dl f c", dl=128)
                            [:, :, 64 * tl:64 * (tl + 1)],
                            in_=rT[:].rearrange("p (f b c) -> p f b c", f=8, b=4)
                            [:, :, b, :])
                    if last:
                        continue
                    dpredT = ap.tile([128, 2048], F16, tag="dpredT")
                    nc.sync.dma_start(out=dpredT[:], in_=cco[:])
                    # ---- backward
                    da = psd.tile([128, 256], F32, tag="da")
                    for f2 in range(8):
                        nc.tensor.matmul(da[:], w1n[:, f2 * 128:(f2 + 1) * 128],
                                         dpredT[:, f2 * 256:(f2 + 1) * 256],
                                         start=(f2 == 0), stop=(f2 == 7))
                    dzT = ap.tile([128, 256], F16, tag="dzT")
                    nc.vector.tensor_mul(dzT[:], da[:], sd[:])
                    dztok = ap.tile([128, 256], F16, tag="dztok")
                    atok = ap.tile([128, 256], F16, tag="atok")
                    for th in range(2):
                        tp = psd.tile([128, 128], F16, tag="da")
                        nc.tensor.transpose(tp[:], dzT[:, th * 128:(th + 1) * 128], eye[:])
                        nc.scalar.copy(dztok[:, th * 128:(th + 1) * 128], tp[:])
                        tp2 = psd.tile([128, 128], F16, tag="da")
                        nc.tensor.transpose(tp2[:], akT[:, th * 128:(th + 1) * 128], eye[:])
                        nc.scalar.activation(atok[:, th * 128:(th + 1) * 128], tp2[:],
                                             AF.Copy, scale=bc[:, 64 + t:65 + t])
                    # dpred token-major via SBUF->SBUF transposing DMA
                    dptok = ap.tile([128, 2048], F16, tag="dptok")
                    for f2 in range(8):
                        for th in range(2):
                            nc.sync.dma_start_transpose(
                                out=dptok[:, th * 1024 + f2 * 128: th * 1024 + (f2 + 1) * 128],
                                in_=dpredT[:, f2 * 256 + th * 128: f2 * 256 + (th + 1) * 128])
                    # ---- gradients in PSUM
                    dw0 = psw.tile([128, 1024], F32, tag="dw0")
                    for f in range(8):
                        for th in range(2):
                            nc.tensor.matmul(
                                dw0[:, f * 128:(f + 1) * 128],
                                ktok[:, th * 1024 + f * 128: th * 1024 + (f + 1) * 128],
                                dztok[:, th * 128:(th + 1) * 128],
                                start=(th == 0), stop=(th == 1))
                    dv = psv.tile([128, 1024], F32, tag="dv")
                    for th in range(2):
                        for nh in range(2):
                            nc.tensor.matmul(
                                dv[:, nh * 512:(nh + 1) * 512],
                                atok[:, th * 128:(th + 1) * 128],
                                dptok[:, th * 1024 + nh * 512: th * 1024 + (nh + 1) * 512],
                                start=(th == 0), stop=(th == 1))
                    # ---- state update (two-term recurrence)
                    c1ap = bc[:, t:t + 1]
                    c2ap = bc[:, 32 + t:33 + t]
                    for (cur, prv, psum, half_t) in (
                        (w_cur, w_prev, dw0, w0h), (v_cur, v_prev, dv, vh)):
                        tmp1 = ap.tile([128, 1024], F32, tag="tmp1")
                        tmp2 = ap.tile([128, 1024], F32, tag="tmp2")
                        nc.scalar.activation(tmp1[:], cur[:], AF.Copy, scale=c1ap)
                        nc.vector.scalar_tensor_tensor(
                            tmp2[:], prv[:], c2ap, tmp1[:], OP.mult, OP.add)
                        nc.vector.tensor_add(prv[:], tmp2[:], psum[:])
                        nc.scalar.copy(half_t[:], prv[:])
                    w_cur, w_prev = w_prev, w_cur
                    v_cur, v_prev = v_prev, v_cur
                    # W1N for step t+1 (scaled by s_{t+1})
                    if t + 1 < NSTEPS - 1:
                        for f2 in range(8):
                            tp = psd.tile([128, 128], F16, tag="da")
                            nc.tensor.transpose(tp[:], vh[:, f2 * 128:(f2 + 1) * 128],
                                                eye[:])
                            nc.scalar.activation(
                                w1n[:, f2 * 128:(f2 + 1) * 128], tp[:], AF.Copy,
                                scale=bc[:, 64 + t + 1:65 + t + 1])

            # ============ PHASE 4: reduce-scatter r, output projection ============
            nc.gpsimd.collective_compute(
                "ReduceScatter", OP.add, replica_groups=RG,
                ins=[rbuf.opt()], outs=[rsh.opt()])
            with (
                tc.tile_pool(name="p4", bufs=2) as p4,
                tc.tile_pool(name="p4w", bufs=1) as p4w,
                tc.tile_pool(name="ps4", bufs=2, space="PSUM") as ps4,
            ):
                rsb = p4w.tile([128, 8192], F16, tag="rsb")
                wosb = p4w.tile([128, 8192], F16, tag="wosb")
                nc.sync.dma_start(out=rsb[:].rearrange("p (f c) -> p f c", f=8),
                                  in_=rsh[:].rearrange("(f dl) c -> dl f c", dl=128))
                nc.sync.dma_start(out=wosb[:].rearrange("p (f c) -> p f c", f=8),
                                  in_=rearr(woT_p))
                for m in range(8):
                    op_ = ps4.tile([128, 1024], F32, tag="op")
                    for dt in range(8):
                        for nh in range(2):
                            nc.tensor.matmul(
                                op_[:, nh * 512:(nh + 1) * 512],
                                rsb[:, dt * 1024 + m * 128: dt * 1024 + (m + 1) * 128],
                                wosb[:, dt * 1024 + nh * 512: dt * 1024 + (nh + 1) * 512],
                                start=(dt == 0), stop=(dt == 7))
                    osb = p4.tile([128, 1024], F32, tag="osb")
                    nc.vector.tensor_copy(osb[:], op_[:])
                    nc.sync.dma_start(out=out_p[m * 128:(m + 1) * 128, :], in_=osb[:])
    return nc


def _prep_inputs(inputs):
    f32 = lambda a: np.ascontiguousarray(np.asarray(a, np.float32))
    f16 = lambda a: np.ascontiguousarray(np.asarray(a, np.float32).astype(np.float16))
    x = f32(inputs["x"])
    Mmet = np.asarray(inputs["Mmet"], np.float32)
    Wk, Wv, Wq, Wo = (np.asarray(inputs[k], np.float32) for k in ["Wk", "Wv", "Wq", "Wo"])
    W0, W1 = np.asarray(inputs["W0"], np.float32), np.asarray(inputs["W1"], np.float32)
    gb = np.concatenate([np.asarray(inputs["bgd"], np.float32),
                        np.asarray(inputs["bgl"], np.float32),
                        np.asarray(inputs["bgm"], np.float32),
                        np.zeros(D, np.float32)]).reshape(1, 4096)
    wgT = np.concatenate([np.asarray(inputs["Wgd"], np.float32).T,
                          np.asarray(inputs["Wgl"], np.float32).T,
                          np.asarray(inputs["Wgm"], np.float32).T], axis=0)
    srow = np.array([[MEM_DECAY / 4096.0], [MEM_LR / 4096.0],
                     [MEM_MOM / 4096.0], [0.0]], np.float32)
    ones1 = np.ones((1, 128), np.float32)
    cmask = np.zeros((128, 2), np.float16)
    for p_ in range(128):
        cmask[p_, p_ // 64] = 1.0
    eye = np.eye(128, dtype=np.float16)
    eyef = np.eye(128, dtype=np.float32)
    gs, gr = np.asarray(inputs["g_store"], np.float32), np.asarray(inputs["g_retr"], np.float32)
    gcol = np.concatenate([gs.reshape(8, 128).T, gr.reshape(8, 128).T], axis=1)
    gcol = np.ascontiguousarray(gcol, dtype=np.float32)

    in_maps = []
    for c in range(NC):
        b, hh = c // 2, c % 2
        blk = slice(128 * c, 128 * (c + 1))
        w0t0 = W0.T[:, blk].reshape(8, 128, 128).transpose(1, 0, 2).reshape(128, 1024)
        v0 = W1.T[blk, :]
        in_maps.append({
            "xs": np.ascontiguousarray(x[b, hh * 1024:(hh + 1) * 1024, :]),
            "mb": f16(Mmet[b]),
            "wkT": f16(Wk.T), "wqT": f16(Wq.T), "wvT": f16(Wv.T), "woT": f16(Wo.T),
            "wgT": f16(wgT), "gb": gb, "srow": srow, "ones1": ones1,
            "cmask": cmask, "eye": eye, "eyef": eyef, "gcol": gcol,
            "w0t0": np.ascontiguousarray(w0t0), "v0": np.ascontiguousarray(v0),
        })
    return in_maps


_CACHE = {}


def _get_built():
    if "nc" not in _CACHE:
        nc = bacc.Bacc("TRN2", target_bir_lowering=False, debug=False)
        build(nc)
        nc.compile()
        _CACHE["nc"] = nc
    return _CACHE["nc"]


def kernel(**inputs) -> np.ndarray:
    nc = _get_built()
    in_maps = _prep_inputs(inputs)
    res = run_bass_kernel_spmd(
        nc, in_maps, core_ids=list(range(NC)),
        trace=bool(int(os.environ.get("DISA_TRACE", "0"))),
    )
    _CACHE["last_results"] = res
    out = np.zeros((B, S, D), np.float32)
    for c in range(NC):
        b, hh = c // 2, c % 2
        out[b, hh * 1024:(hh + 1) * 1024, :] = res.results[c]["out"]
    return out


if __name__ == "__main__":
    import reference
    inputs = {k: np.asarray(v) for k, v in reference.setup_inputs().items()}
    got = kernel(**inputs)
    print("kernel ran; out norm", np.linalg.norm(got))
